# revision 34
# baseline (speedup 1.0000x reference)
"""BERT self-attention (no mask) on 8 TRN2 NeuronCores, head-parallel.

Full inputs in, full output out. Core c computes heads 2c and 2c+1, i.e.
output hidden columns [c*128, (c+1)*128). The host supplies X^T
([H, B*S], f32r) so no on-device transposes of X are needed; projections
consume X^T k-tiles straight from DMA. Matmul operands are float32r
(full-rate near-fp32 streaming). Attention is computed in transposed
layout (scores^T[k, q]) so the softmax denominator comes out of the PV
matmul for free via a ones-column appended to V. The device emits
UNNORMALIZED ctx^T plus denominators; the host divides, transposes, and
adds bv (softmax weights sum to 1, so +bv post-normalization is exact).
Projection (per batch) and attention (previous batch) are interleaved so
TensorE fills the gaps of the ACT-bound exp stream.
"""

import numpy as np

try:
    import concourse.bass as bass
except ImportError:  # toolchain not on sys.path in the caller's environment
    import sys
    sys.path.insert(0, "/opt/trn_rl_repo")
    import concourse.bass as bass
import concourse.bacc as bacc
import concourse.mybir as mybir
import concourse.tile as tile
from concourse.bass_utils import run_bass_kernel_spmd
from concourse.masks import make_identity

F32 = mybir.dt.float32
F32R = mybir.dt.float32r
BF16 = mybir.dt.bfloat16

B = 4
S = 2048
H = 1024
NH = 16
HD = 64
NSEQ = B * S  # 8192
NCORES = 8
CSLICE = H // NCORES  # 128 hidden cols per core = 2 heads
CHUNK = 512  # seq columns per projection chunk
KCH = H // 128  # 8 contraction tiles for projections
KT = S // 128  # 16 key tiles per (b, h)
QC = S // CHUNK  # 4 query chunks per (b, h)
EXPW = 1024  # exp tile width (2 psum banks)
VW = HD + 1  # V' tile width per key tile

_STATE = None


def _build():
    nc = bacc.Bacc("TRN2", target_bir_lowering=False, debug=False,
                   num_devices=NCORES)

    xT = nc.dram_tensor("xT", [H, NSEQ], F32R, kind="ExternalInput").ap()
    ws = {n: nc.dram_tensor(f"w{n}", [H, CSLICE], F32R,
                            kind="ExternalInput").ap()
          for n in "qkv"}
    bs = {n: nc.dram_tensor(f"b{n}", [CSLICE, 1], F32, kind="ExternalInput").ap()
          for n in "qk"}
    # unnormalized ctx^T + denominators: out[b*2+hl, d, s] with d==HD the
    # softmax denominator row; host divides and transposes.
    out = nc.dram_tensor("out", [B * 2, VW, S], F32, kind="ExternalOutput").ap()

    with tile.TileContext(nc) as tc:
        with (
            tc.tile_pool(name="persist", bufs=1) as persist,
            tc.tile_pool(name="qkvt", bufs=2) as qkvt_pool,
            tc.tile_pool(name="xt", bufs=3) as xt_pool,
            tc.tile_pool(name="xtf", bufs=1) as xtf_pool,
            tc.tile_pool(name="vp", bufs=4) as vp_pool,
            tc.tile_pool(name="prob", bufs=12) as prob_pool,
            tc.tile_pool(name="cx", bufs=4) as cx_pool,
            tc.tile_pool(name="ppsum", bufs=2, space="PSUM") as ppsum,
            tc.tile_pool(name="spsum", bufs=2, space="PSUM") as spsum,
            tc.tile_pool(name="cpsum", bufs=2, space="PSUM") as cpsum,
        ):
            # f32r identity: walrus requires transpose operands to share a
            # transfer type when either is 32-bit; f32r streams at 1.5
            # cycles/row vs plain f32's 2.0.
            # 64x64 identity replicated in both partition halves, so
            # transposes of head-1 slices (base partition 64) have a
            # same-base permutation rhs.
            ident2_f = persist.tile([128, HD], F32)
            make_identity(nc, ident2_f[0:HD, :])
            make_identity(nc, ident2_f[HD:128, :])
            ident2 = persist.tile([128, HD], F32R)
            nc.vector.tensor_copy(ident2, ident2_f)
            ones = persist.tile([128, 1], F32)
            nc.vector.memset(ones, 1.0)

            # one DMA per weight matrix: all 8 k-tiles land in a single
            # [128, 8*128] f32r tile via a 3D AP (partition = hid row mod
            # 128, free = [k-tile, out col]). Emitted lazily so chunk 0's
            # X^T loads get the HWDGE pipeline first.
            wt = {}  # weight k-tiles, lhsT layout [k 128, out 128]
            bt = {}

            def load_wq0():
                # tiny first DMA so the very first projection matmul (q,
                # k-tile 0) is gated only by the first X^T k-tile.
                t = persist.tile([128, CSLICE], F32R, tag="wq0", name="wq0")
                nc.scalar.dma_start(t, ws["q"][0:128, :])
                wt["q", 0] = t

            def load_weights():
                for n in "qkv":
                    g0 = 1 if n == "q" else 0
                    g = KCH - g0
                    wall = persist.tile([128, g * CSLICE], F32R,
                                        tag=f"w{n}", name=f"w{n}")
                    nc.scalar.dma_start(
                        wall.rearrange("p (g c) -> p g c", g=g),
                        ws[n][g0 * 128:, :].rearrange(
                            "(g p) c -> p g c", g=g))
                    for kk in range(g0, KCH):
                        wt[n, kk] = wall[:, (kk - g0) * CSLICE:
                                         (kk - g0 + 1) * CSLICE]
                for n in "qk":
                    t = persist.tile([128, 1], F32, tag=f"b{n}", name=f"b{n}")
                    nc.scalar.dma_start(t, bs[n])
                    bt[n] = t

            def alloc_qkvT():
                # per-batch Q^T/K^T/V^T for this core's 2 heads: [128, 2048]
                return {n: qkvt_pool.tile([128, S], F32R,
                                          tag=f"{n}T", name=f"{n}T")
                        for n in "qkv"}

            def project_chunk_a(qkvT, ci, carry, fine=False):
                    # 4 DMAs per chunk (2 k-tiles each, packed along the
                    # free dim via a 3D AP); the very first chunk splits
                    # k-tiles 0 and 1 into their own small DMAs so the
                    # first matmul starts as early as possible.
                    groups = [1, 1, 2, 2, 2] if fine else [2, 2, 2, 2]
                    xts = []
                    k0 = 0
                    for i, nk in enumerate(groups):
                        if nk == 1:
                            xt = xtf_pool.tile([128, CHUNK], F32R,
                                               tag=f"xtf{i}", name="xtf")
                            nc.sync.dma_start(
                                xt, xT[k0 * 128:(k0 + 1) * 128,
                                       ci * CHUNK:(ci + 1) * CHUNK])
                        else:
                            tag = f"xt{i if not fine else i - 1}"
                            xt = xt_pool.tile([128, nk * CHUNK], F32R,
                                              tag=tag, name=tag)
                            src = xT[k0 * 128:(k0 + nk) * 128,
                                     ci * CHUNK:(ci + 1) * CHUNK]
                            nc.sync.dma_start(
                                xt.rearrange("p (g c) -> p g c", g=nk),
                                src.rearrange("(g p) c -> p g c", g=nk))
                        xts.append((xt, k0, nk))
                        k0 += nk
                    carry[ci] = xts

            def project_chunk_b(qkvT, ci, carry, names="qkv"):
                    j = ci % QC
                    xts = carry.pop(ci)
                    for n in names:
                        ps = ppsum.tile([128, CHUNK], F32,
                                        tag="ps", name=f"ps{n}")
                        for xt, k0, nk in xts:
                            for i in range(nk):
                                kk = k0 + i
                                nc.tensor.matmul(
                                    ps, wt[n, kk],
                                    xt[:, i * CHUNK:(i + 1) * CHUNK],
                                    start=(kk == 0), stop=(kk == KCH - 1))
                        dst = qkvT[n][:, j * CHUNK:(j + 1) * CHUNK]
                        if n == "v":
                            nc.vector.tensor_copy(dst, ps)
                        else:
                            nc.vector.tensor_scalar_add(dst, ps, bt[n])

            def prep_v(qkvT, hl):
                # all 16 V^T->V tile transposes go into one borrowed scores
                # psum tile, then a single strided DVE copy scatters them
                # into the VW-strided vp layout.
                p0 = hl * HD
                vT = qkvT["v"][p0:p0 + HD, :]
                vp = vp_pool.tile([128, KT * VW], F32R, tag="vp", name="vp")
                nc.vector.tensor_copy(
                    vp[:, HD::VW], ones.to_broadcast([128, KT]))
                vtp = spsum.tile([128, EXPW], F32, tag="s",
                                 name="vtp").bitcast(F32R)
                for kt in range(KT):
                    nc.tensor.transpose(
                        vtp[:, kt * HD:(kt + 1) * HD],
                        vT[:, kt * 128:(kt + 1) * 128],
                        ident2[p0:p0 + HD, :])
                nc.vector.tensor_copy(
                    vp.rearrange("p (kt d) -> p kt d", kt=KT)[:, :, 0:HD],
                    vtp.rearrange("p (kt d) -> p kt d", kt=KT))
                return vp

            def attend_qc(qkvT, b, hl, vp, qc):
                    p0 = hl * HD      # partition offset of this head
                    qT = qkvT["q"][p0:p0 + HD, :]
                    kTt = qkvT["k"][p0:p0 + HD, :]
                    ctx_ps = cpsum.tile([VW, CHUNK], F32,
                                        tag="ctx", name="ctx")
                    rhs_q = qT[:, qc * CHUNK:(qc + 1) * CHUNK]
                    for kp in range(KT // 2):  # pairs of key tiles
                        s_ps = spsum.tile([128, EXPW], F32, tag="s", name="s")
                        with tc.high_priority(offset=150):
                            for half in range(2):
                                kt = kp * 2 + half
                                nc.tensor.matmul(
                                    s_ps[:, half * CHUNK:(half + 1) * CHUNK],
                                    kTt[:, kt * 128:(kt + 1) * 128],
                                    rhs_q, start=True, stop=True)
                        pr = prob_pool.tile([128, EXPW], F32R,
                                            tag="pr", name="pr")
                        nc.scalar.activation(
                            pr, s_ps, mybir.ActivationFunctionType.Exp,
                            scale=1.0 / np.sqrt(float(HD)))
                        for half in range(2):
                            kt = kp * 2 + half
                            nc.tensor.matmul(
                                ctx_ps,
                                vp[:, kt * VW:(kt + 1) * VW],
                                pr[:, half * CHUNK:(half + 1) * CHUNK],
                                start=(kt == 0), stop=(kt == KT - 1))
                    cx = cx_pool.tile([VW, CHUNK], F32, tag="cx", name="cx")
                    with tc.high_priority(offset=150):
                        nc.vector.tensor_copy(cx, ctx_ps)
                    nc.sync.dma_start(
                        out[b * 2 + hl, :, qc * CHUNK:(qc + 1) * CHUNK], cx)

            def att_steps(qkvT, b, hl, vp):
                return [lambda qc=qc: attend_qc(qkvT, b, hl, vp, qc)
                        for qc in range(QC)]

            # software-pipelined emission: projection + V'-prep of batch
            # b+1 are emitted between the ACT-bound attention q-chunks of
            # batch b, giving the list scheduler adjacent independent work
            vps = {}
            qkvTs = {}
            carry = {}
            # warm the PE p-state while the first DMAs are in flight: cheap
            # dummy matmuls on the identity (no DMA dependency), sized to
            # end roughly when the first X^T k-tile lands. They run on the
            # ctx psum ring, which attention won't touch for ~30us.
            for i in range(26):
                wps = cpsum.tile([VW, CHUNK], F32, tag="ctx", name="warm")
                nc.tensor.matmul(wps[0:HD, 0:HD], ident2[0:HD, :],
                                 ident2[0:HD, :], start=True, stop=True)

            qkvTs[0] = alloc_qkvT()
            load_wq0()
            project_chunk_a(qkvTs[0], 0, carry, fine=True)
            load_weights()
            project_chunk_a(qkvTs[0], 1, carry)
            project_chunk_b(qkvTs[0], 0, carry)
            project_chunk_a(qkvTs[0], 2, carry)
            project_chunk_b(qkvTs[0], 1, carry)
            project_chunk_a(qkvTs[0], 3, carry)
            project_chunk_b(qkvTs[0], 2, carry)
            project_chunk_b(qkvTs[0], 3, carry)
            vps[0, 0] = prep_v(qkvTs[0], 0)
            vps[0, 1] = prep_v(qkvTs[0], 1)
            for b in range(B):
                if b == B - 1:
                    # last batch: no next-batch projection filler exists, so
                    # Q was held back (only K/V were projected ahead); emit
                    # Q chunk projections just-in-time, qc-major, as
                    # TensorE filler for the ACT-bound exp stream.
                    qkvT = qkvTs[b]

                    def qjit(qc, qkvT=qkvT, b=b):
                        project_chunk_b(qkvT, b * QC + qc, carry, names="q")

                    def aqc(hl, qc, qkvT=qkvT, b=b):
                        return lambda: attend_qc(qkvT, b, hl, vps[b, hl], qc)

                    def adma(qc, qkvT=qkvT, b=b):
                        return lambda: project_chunk_a(qkvT, b * QC + qc,
                                                       carry)

                    att = [
                        adma(1), lambda: qjit(0),
                        adma(2), lambda: qjit(1),
                        aqc(0, 0), aqc(1, 0),
                        adma(3), lambda: qjit(2),
                        aqc(0, 1), aqc(1, 1),
                        lambda: qjit(3),
                        aqc(0, 2), aqc(1, 2),
                        aqc(0, 3), aqc(1, 3),
                    ]
                else:
                    att = (att_steps(qkvTs[b], b, 0, vps[b, 0])
                           + att_steps(qkvTs[b], b, 1, vps[b, 1]))
                nxt = []
                if b + 1 < B:
                    names = "kv" if b + 1 == B - 1 else "qkv"
                    qkvTs[b + 1] = alloc_qkvT()
                    for ci in range(QC * (b + 1), QC * (b + 2)):
                        nxt.append(lambda ci=ci: project_chunk_a(
                            qkvTs[b + 1], ci, carry))
                        nxt.append(lambda ci=ci, names=names: project_chunk_b(
                            qkvTs[b + 1], ci, carry, names=names))
                    nxt.append(lambda: vps.__setitem__(
                        (b + 1, 0), prep_v(qkvTs[b + 1], 0)))
                    nxt.append(lambda: vps.__setitem__(
                        (b + 1, 1), prep_v(qkvTs[b + 1], 1)))
                    if b + 1 == B - 1:
                        nxt.append(lambda: project_chunk_a(
                            qkvTs[b + 1], QC * (b + 1), carry))
                # 8 att steps, up to 10 nxt steps: round-robin interleave
                order = list(att[:2])
                ai, ni = 2, 0
                while ai < len(att) or ni < len(nxt):
                    if ai < len(att):
                        order.append(att[ai]); ai += 1
                    if ni < len(nxt):
                        order.append(nxt[ni]); ni += 1
                    if ni < len(nxt) and len(nxt) - ni > len(att) - ai:
                        order.append(nxt[ni]); ni += 1
                for step in order:
                    step()

    nc.compile()
    return nc


def _get_nc():
    global _STATE
    if _STATE is None:
        _STATE = _build()
    return _STATE


def _in_maps(inputs):
    x = np.asarray(inputs["hidden_states"], dtype=np.float32).reshape(NSEQ, H)
    xTf = np.ascontiguousarray(x.T)  # [H, NSEQ]
    maps = []
    for c in range(NCORES):
        sl = slice(c * CSLICE, (c + 1) * CSLICE)
        m = {"xT": xTf}
        for n, wkey in (("q", "Wq"), ("k", "Wk"), ("v", "Wv")):
            m[f"w{n}"] = np.ascontiguousarray(
                np.asarray(inputs[wkey], dtype=np.float32)[:, sl])
        for n, bkey in (("q", "bq"), ("k", "bk")):
            m[f"b{n}"] = np.ascontiguousarray(
                np.asarray(inputs[bkey], dtype=np.float32)[sl].reshape(
                    CSLICE, 1))
        maps.append(m)
    return maps


def _assemble(results, inputs):
    bv = np.asarray(inputs["bv"], dtype=np.float32)
    full = np.empty((B, S, H), dtype=np.float32)
    for c in range(NCORES):
        o = results[c]["out"].reshape(B, 2, VW, S)
        ctx = o[:, :, :HD, :] / o[:, :, HD:HD + 1, :]  # [B, 2, HD, S]
        # -> [B, S, 2*HD]
        full[:, :, c * CSLICE:(c + 1) * CSLICE] = (
            ctx.transpose(0, 3, 1, 2).reshape(B, S, 2 * HD))
    full += bv.reshape(1, 1, H)
    return full


def _run(inputs, trace=False):
    nc = _get_nc()
    maps = _in_maps(inputs)
    last_err = None
    for attempt in range(3):
        try:
            res = run_bass_kernel_spmd(nc, maps,
                                       core_ids=list(range(NCORES)),
                                       trace=trace)
            return _assemble(res.results, inputs), res
        except Exception as e:  # transient NRT_EXEC_UNIT_UNRECOVERABLE
            last_err = e
            if attempt < 2:
                import time
                time.sleep(2.0)
    raise last_err


def kernel(**inputs):
    out, _ = _run(inputs, trace=False)
    return out


def run_traced(**inputs):
    out, res = _run(inputs, trace=True)
    return out, res


# revision 37
# speedup vs baseline: 1.0009x; 1.0009x over previous
"""BERT self-attention (no mask) on 8 TRN2 NeuronCores, head-parallel.

Full inputs in, full output out. Core c computes heads 2c and 2c+1, i.e.
output hidden columns [c*128, (c+1)*128). The host supplies X^T
([H, B*S], f32r) so no on-device transposes of X are needed; projections
consume X^T k-tiles straight from DMA. Matmul operands are float32r
(full-rate near-fp32 streaming). Attention is computed in transposed
layout (scores^T[k, q]) so the softmax denominator comes out of the PV
matmul for free via a ones-column appended to V. The device emits
UNNORMALIZED ctx^T plus denominators; the host divides, transposes, and
adds bv (softmax weights sum to 1, so +bv post-normalization is exact).
Projection (per batch) and attention (previous batch) are interleaved so
TensorE fills the gaps of the ACT-bound exp stream.
"""

import numpy as np

try:
    import concourse.bass as bass
except ImportError:  # toolchain not on sys.path in the caller's environment
    import sys
    sys.path.insert(0, "/opt/trn_rl_repo")
    import concourse.bass as bass
import concourse.bacc as bacc
import concourse.mybir as mybir
import concourse.tile as tile
from concourse.bass_utils import run_bass_kernel_spmd
from concourse.masks import make_identity

F32 = mybir.dt.float32
F32R = mybir.dt.float32r
BF16 = mybir.dt.bfloat16

B = 4
S = 2048
H = 1024
NH = 16
HD = 64
NSEQ = B * S  # 8192
NCORES = 8
CSLICE = H // NCORES  # 128 hidden cols per core = 2 heads
CHUNK = 512  # seq columns per projection chunk
KCH = H // 128  # 8 contraction tiles for projections
KT = S // 128  # 16 key tiles per (b, h)
QC = S // CHUNK  # 4 query chunks per (b, h)
EXPW = 1024  # exp tile width (2 psum banks)
VW = HD + 1  # V' tile width per key tile

_STATE = None


def _build():
    nc = bacc.Bacc("TRN2", target_bir_lowering=False, debug=False,
                   num_devices=NCORES)

    xT = nc.dram_tensor("xT", [H, NSEQ], F32R, kind="ExternalInput").ap()
    ws = {n: nc.dram_tensor(f"w{n}", [H, CSLICE], F32R,
                            kind="ExternalInput").ap()
          for n in "qkv"}
    bs = {n: nc.dram_tensor(f"b{n}", [CSLICE, 1], F32, kind="ExternalInput").ap()
          for n in "qk"}
    # unnormalized ctx^T + denominators: out[b*2+hl, d, s] with d==HD the
    # softmax denominator row; host divides and transposes.
    out = nc.dram_tensor("out", [B * 2, VW, S], F32, kind="ExternalOutput").ap()

    with tile.TileContext(nc) as tc:
        with (
            tc.tile_pool(name="persist", bufs=1) as persist,
            tc.tile_pool(name="qkvt", bufs=2) as qkvt_pool,
            tc.tile_pool(name="xt", bufs=3) as xt_pool,
            tc.tile_pool(name="vp", bufs=4) as vp_pool,
            tc.tile_pool(name="prob", bufs=12) as prob_pool,
            tc.tile_pool(name="cx", bufs=4) as cx_pool,
            tc.tile_pool(name="ppsum", bufs=2, space="PSUM") as ppsum,
            tc.tile_pool(name="spsum", bufs=2, space="PSUM") as spsum,
            tc.tile_pool(name="cpsum", bufs=2, space="PSUM") as cpsum,
        ):
            # f32r identity: walrus requires transpose operands to share a
            # transfer type when either is 32-bit; f32r streams at 1.5
            # cycles/row vs plain f32's 2.0.
            # 64x64 identity replicated in both partition halves, so
            # transposes of head-1 slices (base partition 64) have a
            # same-base permutation rhs.
            ident2_f = persist.tile([128, HD], F32)
            make_identity(nc, ident2_f[0:HD, :])
            make_identity(nc, ident2_f[HD:128, :])
            ident2 = persist.tile([128, HD], F32R)
            nc.vector.tensor_copy(ident2, ident2_f)
            ones = persist.tile([128, 1], F32)
            nc.vector.memset(ones, 1.0)

            # one DMA per weight matrix: all 8 k-tiles land in a single
            # [128, 8*128] f32r tile via a 3D AP (partition = hid row mod
            # 128, free = [k-tile, out col]). Emitted lazily so chunk 0's
            # X^T loads get the HWDGE pipeline first.
            wt = {}  # weight k-tiles, lhsT layout [k 128, out 128]
            bt = {}

            def load_weights():
                for n in "qkv":
                    wall = persist.tile([128, KCH * CSLICE], F32R,
                                        tag=f"w{n}", name=f"w{n}")
                    nc.scalar.dma_start(
                        wall.rearrange("p (g c) -> p g c", g=KCH),
                        ws[n].rearrange("(g p) c -> p g c", g=KCH))
                    for kk in range(KCH):
                        wt[n, kk] = wall[:, kk * CSLICE:(kk + 1) * CSLICE]
                for n in "qk":
                    t = persist.tile([128, 1], F32, tag=f"b{n}", name=f"b{n}")
                    nc.scalar.dma_start(t, bs[n])
                    bt[n] = t

            def alloc_qkvT():
                # per-batch Q^T/K^T/V^T for this core's 2 heads: [128, 2048]
                return {n: qkvt_pool.tile([128, S], F32R,
                                          tag=f"{n}T", name=f"{n}T")
                        for n in "qkv"}

            def project_chunk_a(qkvT, ci, carry):
                    # 4 DMAs per chunk: each loads 2 contraction tiles
                    # [128, CHUNK] packed along the free dim via a 3D AP.
                    xts = []
                    for g in range(4):
                        xt = xt_pool.tile([128, 2 * CHUNK], F32R,
                                          tag=f"xt{g}", name=f"xt{g}")
                        src = xT[g * 256:(g + 1) * 256,
                                 ci * CHUNK:(ci + 1) * CHUNK]
                        nc.sync.dma_start(
                            xt.rearrange("p (g c) -> p g c", g=2),
                            src.rearrange("(g p) c -> p g c", g=2))
                        xts.append(xt)
                    carry[ci] = xts

            def project_chunk_b(qkvT, ci, carry, names="qkv"):
                    j = ci % QC
                    xts = carry.pop(ci)
                    for n in names:
                        ps = ppsum.tile([128, CHUNK], F32,
                                        tag="ps", name=f"ps{n}")
                        for kk in range(KCH):
                            nc.tensor.matmul(
                                ps, wt[n, kk],
                                xts[kk // 2][:, (kk % 2) * CHUNK:
                                             (kk % 2 + 1) * CHUNK],
                                start=(kk == 0), stop=(kk == KCH - 1))
                        dst = qkvT[n][:, j * CHUNK:(j + 1) * CHUNK]
                        if n == "v":
                            nc.vector.tensor_copy(dst, ps)
                        else:
                            nc.vector.tensor_scalar_add(dst, ps, bt[n])

            def prep_v(qkvT, hl):
                # all 16 V^T->V tile transposes go into one borrowed scores
                # psum tile, then a single strided DVE copy scatters them
                # into the VW-strided vp layout.
                p0 = hl * HD
                vT = qkvT["v"][p0:p0 + HD, :]
                vp = vp_pool.tile([128, KT * VW], F32R, tag="vp", name="vp")
                nc.vector.tensor_copy(
                    vp[:, HD::VW], ones.to_broadcast([128, KT]))
                vtp = spsum.tile([128, EXPW], F32, tag="s",
                                 name="vtp").bitcast(F32R)
                for kt in range(KT):
                    nc.tensor.transpose(
                        vtp[:, kt * HD:(kt + 1) * HD],
                        vT[:, kt * 128:(kt + 1) * 128],
                        ident2[p0:p0 + HD, :])
                nc.vector.tensor_copy(
                    vp.rearrange("p (kt d) -> p kt d", kt=KT)[:, :, 0:HD],
                    vtp.rearrange("p (kt d) -> p kt d", kt=KT))
                return vp

            def attend_qc(qkvT, b, hl, vp, qc, filler=None):
                    p0 = hl * HD      # partition offset of this head
                    qT = qkvT["q"][p0:p0 + HD, :]
                    kTt = qkvT["k"][p0:p0 + HD, :]
                    ctx_ps = cpsum.tile([VW, CHUNK], F32,
                                        tag="ctx", name="ctx")
                    rhs_q = qT[:, qc * CHUNK:(qc + 1) * CHUNK]
                    for kp in range(KT // 2):  # pairs of key tiles
                        s_ps = spsum.tile([128, EXPW], F32, tag="s", name="s")
                        with tc.high_priority(offset=150):
                            for half in range(2):
                                kt = kp * 2 + half
                                nc.tensor.matmul(
                                    s_ps[:, half * CHUNK:(half + 1) * CHUNK],
                                    kTt[:, kt * 128:(kt + 1) * 128],
                                    rhs_q, start=True, stop=True)
                        pr = prob_pool.tile([128, EXPW], F32R,
                                            tag="pr", name="pr")
                        nc.scalar.activation(
                            pr, s_ps, mybir.ActivationFunctionType.Exp,
                            scale=1.0 / np.sqrt(float(HD)))
                        for half in range(2):
                            kt = kp * 2 + half
                            nc.tensor.matmul(
                                ctx_ps,
                                vp[:, kt * VW:(kt + 1) * VW],
                                pr[:, half * CHUNK:(half + 1) * CHUNK],
                                start=(kt == 0), stop=(kt == KT - 1))
                        if filler is not None:
                            # weave one pending projection micro-op into
                            # the ACT-bound kp cadence
                            next(filler, None)
                    cx = cx_pool.tile([VW, CHUNK], F32, tag="cx", name="cx")
                    with tc.high_priority(offset=150):
                        nc.vector.tensor_copy(cx, ctx_ps)
                    nc.sync.dma_start(
                        out[b * 2 + hl, :, qc * CHUNK:(qc + 1) * CHUNK], cx)

            def att_steps(qkvT, b, hl, vp):
                return [lambda qc=qc: attend_qc(qkvT, b, hl, vp, qc)
                        for qc in range(QC)]

            # software-pipelined emission: projection + V'-prep of batch
            # b+1 are emitted between the ACT-bound attention q-chunks of
            # batch b, giving the list scheduler adjacent independent work
            vps = {}
            qkvTs = {}
            carry = {}
            qkvTs[0] = alloc_qkvT()
            project_chunk_a(qkvTs[0], 0, carry)
            load_weights()
            project_chunk_b(qkvTs[0], 0, carry)
            for ci in range(1, QC):
                project_chunk_a(qkvTs[0], ci, carry)
                project_chunk_b(qkvTs[0], ci, carry)
            vps[0, 0] = prep_v(qkvTs[0], 0)
            vps[0, 1] = prep_v(qkvTs[0], 1)
            for b in range(B):
                if b == B - 1:
                    # last batch: no next-batch projection filler exists, so
                    # Q was held back (only K/V were projected ahead); emit
                    # Q chunk projections just-in-time, qc-major, as
                    # TensorE filler for the ACT-bound exp stream.
                    qkvT = qkvTs[b]

                    def qjit(qc, qkvT=qkvT, b=b):
                        project_chunk_b(qkvT, b * QC + qc, carry, names="q")

                    def qjit_micro(qc, qkvT=qkvT, b=b):
                        # emits one q-projection micro-op per next() so the
                        # work weaves between attention kp-pairs
                        ci = b * QC + qc
                        j = ci % QC
                        xts = carry.pop(ci)
                        ps = ppsum.tile([128, CHUNK], F32,
                                        tag="ps", name="psq")
                        for kk in range(KCH):
                            nc.tensor.matmul(
                                ps, wt["q", kk],
                                xts[kk // 2][:, (kk % 2) * CHUNK:
                                             (kk % 2 + 1) * CHUNK],
                                start=(kk == 0), stop=(kk == KCH - 1))
                            yield
                        nc.vector.tensor_scalar_add(
                            qkvT["q"][:, j * CHUNK:(j + 1) * CHUNK],
                            ps, bt["q"])
                        yield

                    import itertools
                    fill = itertools.chain(
                        qjit_micro(1), qjit_micro(2), qjit_micro(3))

                    def aqc(hl, qc, qkvT=qkvT, b=b):
                        return lambda: attend_qc(qkvT, b, hl, vps[b, hl],
                                                 qc, filler=fill)

                    def adma(qc, qkvT=qkvT, b=b):
                        return lambda: project_chunk_a(qkvT, b * QC + qc,
                                                       carry)

                    att = [
                        adma(1), lambda: qjit(0),
                        adma(2),
                        aqc(0, 0), aqc(1, 0),
                        adma(3),
                        aqc(0, 1), aqc(1, 1),
                        aqc(0, 2), aqc(1, 2),
                        aqc(0, 3), aqc(1, 3),
                    ]
                else:
                    att = (att_steps(qkvTs[b], b, 0, vps[b, 0])
                           + att_steps(qkvTs[b], b, 1, vps[b, 1]))
                nxt = []
                if b + 1 < B:
                    names = "kv" if b + 1 == B - 1 else "qkv"
                    qkvTs[b + 1] = alloc_qkvT()
                    for ci in range(QC * (b + 1), QC * (b + 2)):
                        nxt.append(lambda ci=ci: project_chunk_a(
                            qkvTs[b + 1], ci, carry))
                        nxt.append(lambda ci=ci, names=names: project_chunk_b(
                            qkvTs[b + 1], ci, carry, names=names))
                    nxt.append(lambda: vps.__setitem__(
                        (b + 1, 0), prep_v(qkvTs[b + 1], 0)))
                    nxt.append(lambda: vps.__setitem__(
                        (b + 1, 1), prep_v(qkvTs[b + 1], 1)))
                    if b + 1 == B - 1:
                        nxt.append(lambda: project_chunk_a(
                            qkvTs[b + 1], QC * (b + 1), carry))
                # 8 att steps, up to 10 nxt steps: round-robin interleave
                order = list(att[:2])
                ai, ni = 2, 0
                while ai < len(att) or ni < len(nxt):
                    if ai < len(att):
                        order.append(att[ai]); ai += 1
                    if ni < len(nxt):
                        order.append(nxt[ni]); ni += 1
                    if ni < len(nxt) and len(nxt) - ni > len(att) - ai:
                        order.append(nxt[ni]); ni += 1
                for step in order:
                    step()

    nc.compile()
    return nc


def _get_nc():
    global _STATE
    if _STATE is None:
        _STATE = _build()
    return _STATE


def _in_maps(inputs):
    x = np.asarray(inputs["hidden_states"], dtype=np.float32).reshape(NSEQ, H)
    xTf = np.ascontiguousarray(x.T)  # [H, NSEQ]
    maps = []
    for c in range(NCORES):
        sl = slice(c * CSLICE, (c + 1) * CSLICE)
        m = {"xT": xTf}
        for n, wkey in (("q", "Wq"), ("k", "Wk"), ("v", "Wv")):
            m[f"w{n}"] = np.ascontiguousarray(
                np.asarray(inputs[wkey], dtype=np.float32)[:, sl])
        for n, bkey in (("q", "bq"), ("k", "bk")):
            m[f"b{n}"] = np.ascontiguousarray(
                np.asarray(inputs[bkey], dtype=np.float32)[sl].reshape(
                    CSLICE, 1))
        maps.append(m)
    return maps


def _assemble(results, inputs):
    bv = np.asarray(inputs["bv"], dtype=np.float32)
    full = np.empty((B, S, H), dtype=np.float32)
    for c in range(NCORES):
        o = results[c]["out"].reshape(B, 2, VW, S)
        ctx = o[:, :, :HD, :] / o[:, :, HD:HD + 1, :]  # [B, 2, HD, S]
        # -> [B, S, 2*HD]
        full[:, :, c * CSLICE:(c + 1) * CSLICE] = (
            ctx.transpose(0, 3, 1, 2).reshape(B, S, 2 * HD))
    full += bv.reshape(1, 1, H)
    return full


def _run(inputs, trace=False):
    nc = _get_nc()
    maps = _in_maps(inputs)
    last_err = None
    for attempt in range(3):
        try:
            res = run_bass_kernel_spmd(nc, maps,
                                       core_ids=list(range(NCORES)),
                                       trace=trace)
            return _assemble(res.results, inputs), res
        except Exception as e:  # transient NRT_EXEC_UNIT_UNRECOVERABLE
            last_err = e
            if attempt < 2:
                import time
                time.sleep(2.0)
    raise last_err


def kernel(**inputs):
    out, _ = _run(inputs, trace=False)
    return out


def run_traced(**inputs):
    out, res = _run(inputs, trace=True)
    return out, res
